# revision 27
# baseline (speedup 1.0000x reference)
"""Trainium2 Bass kernel for nn_AttenLayer (sparse graph attention).

Reference computation (B=4, S=512, N=4096, D=512):
    xt = x.transpose(0, 2, 1)                  # (B, N, S)
    Q = xt @ Wq + bq; K = xt @ Wk + bk; V = xt @ Wv + bv
    E = Q @ K^T                                # (B, N, N)
    E = where(adj > 0, E, -9e15)
    atten = softmax(E, axis=1)                 # over the query-node axis!
    V' = (atten @ V) / sqrt(512)

Sharding: 8 cores = 4 batches x 2 key-node (m) halves. Each core works in
the transposed score layout scoresT[m, n] = K @ Q^T so the softmax axis n
is the free axis:
    P1: QT[d, n] full and V[m_shard, d] projections (PE, fp32r);
        KT[d, m] is produced on the fly in P2 per 512-wide m chunk.
    P2 per m_tile: scoresT into PSUM; masked = scores + 9e15*(adjT-1)
        (DVE scalar_tensor_tensor, int8 mask); expE = exp(masked) with
        fused row sums (ACT accum_out, in place); atten_T = expE / rowsum
        (DVE/ACT split, in place); DMA out.
    P3: V'_partial = atten_T^T @ V_scaled, re-reading atten_T from HBM.
Host reassembles: atten[b] = vstack(two m-half atten_T parts).T and
V'[b] = sum of the two partial V'. The 1/sqrt(512) scale is folded into
the V projection on-chip. Each core receives xb with its own m-half
permuted to the front; the host undoes the n-axis permutation on gather.
"""

import math

import numpy as np

import concourse.bacc as bacc
import concourse.mybir as mybir
import concourse.tile as tile

F32 = mybir.dt.float32
F32R = mybir.dt.float32r
I8 = mybir.dt.int8
AF = mybir.ActivationFunctionType
ALU = mybir.AluOpType

NEGBIG = 9.0e15


def build_program(S=512, N=4096, D=512, MSH=2048, num_devices=8, with_bias=True):
    """One-core program; run SPMD on all cores with per-core input slices."""
    ST, DT = S // 128, D // 128
    NCH = N // 512          # n chunks of 512 (psum bank width)
    MT = MSH // 128         # m tiles
    MCH = MSH // 512        # m chunks (KT projection granularity)
    HALF = N // 2           # exp spans
    NRM_DVE = max(512, (3 * N // 8) // 512 * 512)  # DVE-normalized prefix
    VSCALE = 1.0 / math.sqrt(512.0)

    nc = bacc.Bacc("TRN2", target_bir_lowering=False, debug=False,
                   num_devices=num_devices)

    xb = nc.dram_tensor("xb", [S, N], F32R, kind="ExternalInput")
    wq = nc.dram_tensor("wq", [S, D], F32R, kind="ExternalInput")
    wk = nc.dram_tensor("wk", [S, D], F32R, kind="ExternalInput")
    wv = nc.dram_tensor("wv", [S, D], F32R, kind="ExternalInput")
    bq = nc.dram_tensor("bq", [D], F32, kind="ExternalInput")
    bk = nc.dram_tensor("bk", [D], F32, kind="ExternalInput")
    bv = nc.dram_tensor("bv", [D], F32, kind="ExternalInput")
    adjm1 = nc.dram_tensor("adjm1", [MSH, N], I8, kind="ExternalInput")
    attn = nc.dram_tensor("attn", [MSH, N], F32, kind="ExternalOutput")
    vp = nc.dram_tensor("vp", [N, D], F32, kind="ExternalOutput")

    with tile.TileContext(nc) as tc:
        with (
            tc.tile_pool(name="pers", bufs=1) as pers,
            tc.tile_pool(name="xbp", bufs=2) as xbp,
            tc.tile_pool(name="ktp", bufs=2) as ktp,
            tc.tile_pool(name="rows", bufs=2) as rows,
            tc.tile_pool(name="masks", bufs=2) as masks,
            tc.tile_pool(name="atiles", bufs=6) as atiles,
            tc.tile_pool(name="stats", bufs=6) as stats,
            tc.tile_pool(name="pjps", bufs=2, space="PSUM") as pjps,
            tc.tile_pool(name="scps", bufs=6, space="PSUM") as scps,
        ):
            # ---- constants / weights -------------------------------------
            wq_t = pers.tile([128, ST, D], F32R, tag="wq")
            wk_t = pers.tile([128, ST, D], F32R, tag="wk")
            wv_t = pers.tile([128, ST, D], F32R, tag="wv")
            nc.sync.dma_start(out=wq_t[:], in_=wq.rearrange("(st p) d -> p st d", p=128))
            nc.sync.dma_start(out=wk_t[:], in_=wk.rearrange("(st p) d -> p st d", p=128))
            nc.sync.dma_start(out=wv_t[:], in_=wv.rearrange("(st p) d -> p st d", p=128))
            if with_bias:
                bq_r = pers.tile([1, D], F32R, tag="bq")
                bk_r = pers.tile([1, D], F32R, tag="bk")
                bv_r = pers.tile([1, D], F32R, tag="bv")
                nc.sync.dma_start(out=bq_r[:], in_=bq[None, :].bitcast(F32R))
                nc.sync.dma_start(out=bk_r[:], in_=bk[None, :].bitcast(F32R))
                nc.sync.dma_start(out=bv_r[:], in_=bv[None, :].bitcast(F32R))
                ones_f = pers.tile([1, 512], F32, tag="ones_f")
                nc.vector.memset(ones_f[:], 1.0)
                ones_t = pers.tile([1, 128], F32R, tag="ones")
                nc.vector.tensor_copy(ones_t[:], ones_f[:, 0:128])
                ones512 = pers.tile([1, 512], F32R, tag="ones512")
                nc.vector.tensor_copy(ones512[:], ones_f[:])

            qt_t = pers.tile([128, DT, N], F32R, tag="qt")
            v_t = pers.tile([128, MT, D], F32R, tag="vt")

            # ---- P1: QT and V projections --------------------------------
            # QT[d, n] = Wq^T @ xb + bq: lhsT = wq[s, d], rhs = xb[s, n]
            for nch in range(NCH):
                xch = xbp.tile([128, ST, 512], F32R, tag="xch", name="xq")
                nc.sync.dma_start(
                    out=xch[:],
                    in_=xb[:, nch * 512:(nch + 1) * 512].rearrange(
                        "(st p) n -> p st n", p=128),
                )
                for dt in range(DT):
                    ps = scps.tile([128, 512], F32, tag="sc", name="psq")
                    if with_bias:
                        nc.tensor.matmul(
                            ps[:], bq_r[0:1, dt * 128:(dt + 1) * 128],
                            ones512[0:1, :], start=True, stop=False,
                        )
                    for st in range(ST):
                        nc.tensor.matmul(
                            ps[:], wq_t[:, st, dt * 128:(dt + 1) * 128],
                            xch[:, st, :],
                            start=(st == 0 and not with_bias),
                            stop=(st == ST - 1),
                        )
                    nc.scalar.activation(
                        qt_t[:, dt, nch * 512:(nch + 1) * 512], ps[:], AF.Copy,
                    )

            # V[m, d] = (xb_m^T @ Wv + bv) / sqrt(512); m = first MSH cols
            for mch in range(MCH):
                xch = xbp.tile([128, ST, 512], F32R, tag="xch", name="xv")
                nc.sync.dma_start(
                    out=xch[:],
                    in_=xb[:, mch * 512:(mch + 1) * 512].rearrange(
                        "(st p) n -> p st n", p=128),
                )
                for mt_in in range(4):
                    mt = mch * 4 + mt_in
                    ps = scps.tile([128, D], F32, tag="sc", name="psv")
                    if with_bias:
                        nc.tensor.matmul(
                            ps[:], ones_t[0:1, :], bv_r[0:1, :],
                            start=True, stop=False,
                        )
                    for st in range(ST):
                        nc.tensor.matmul(
                            ps[:], xch[:, st, mt_in * 128:(mt_in + 1) * 128],
                            wv_t[:, st, :],
                            start=(st == 0 and not with_bias),
                            stop=(st == ST - 1),
                        )
                    nc.scalar.activation(v_t[:, mt, :], ps[:], AF.Copy, scale=VSCALE)

            # ---- P2: scores -> masked softmax (m-outer) ------------------
            def kt_build(mch):
                # KT[d, m_chunk] = Wk^T @ xb_m + bk, produced per 512 chunk
                xch = xbp.tile([128, ST, 512], F32R, tag="xch", name="xk")
                nc.sync.dma_start(
                    out=xch[:],
                    in_=xb[:, mch * 512:(mch + 1) * 512].rearrange(
                        "(st p) n -> p st n", p=128),
                )
                kt_c = ktp.tile([128, DT, 512], F32R, tag="ktc", name="ktc")
                for dt in range(DT):
                    ps = pjps.tile([128, 512], F32, tag="pj", name="psk")
                    if with_bias:
                        nc.tensor.matmul(
                            ps[:], bk_r[0:1, dt * 128:(dt + 1) * 128],
                            ones512[0:1, :], start=True, stop=False,
                        )
                    for st in range(ST):
                        nc.tensor.matmul(
                            ps[:], wk_t[:, st, dt * 128:(dt + 1) * 128],
                            xch[:, st, :],
                            start=(st == 0 and not with_bias),
                            stop=(st == ST - 1),
                        )
                    nc.scalar.activation(kt_c[:, dt, :], ps[:], AF.Copy)
                return kt_c

            kt_cur = kt_build(0)
            for mch in range(MCH):
                kt_c = kt_cur
                for mt_in in range(4):
                    mt = mch * 4 + mt_in
                    if mt_in == 1 and mch + 1 < MCH:
                        kt_cur = kt_build(mch + 1)
                    msl = slice(mt_in * 128, (mt_in + 1) * 128)
                    mk = masks.tile([128, N], I8, tag="mk", name="mk")
                    nc.sync.dma_start(
                        out=mk[:], in_=adjm1[mt * 128:(mt + 1) * 128, :])
                    row = rows.tile([128, N], F32, tag="row", name="row")
                    for nch in range(NCH):
                        ps = scps.tile([128, 512], F32, tag="sc", name="sc")
                        for dt in range(DT):
                            nc.tensor.matmul(
                                ps[:], kt_c[:, dt, msl],
                                qt_t[:, dt, nch * 512:(nch + 1) * 512],
                                start=(dt == 0), stop=(dt == DT - 1),
                            )
                        nc.vector.scalar_tensor_tensor(
                            out=row[:, nch * 512:(nch + 1) * 512],
                            in0=mk[:, nch * 512:(nch + 1) * 512], scalar=NEGBIG,
                            in1=ps[:], op0=ALU.mult, op1=ALU.add,
                        )
                    rs_a = stats.tile([128, 1], F32, tag="rsa", name="rsa")
                    rs_b = stats.tile([128, 1], F32, tag="rsb", name="rsb")
                    nc.scalar.activation(
                        row[:, 0:HALF], row[:, 0:HALF], AF.Exp,
                        accum_out=rs_a[:],
                    )
                    nc.scalar.activation(
                        row[:, HALF:], row[:, HALF:], AF.Exp,
                        accum_out=rs_b[:],
                    )
                    r_t = stats.tile([128, 1], F32, tag="r", name="r")
                    nc.vector.tensor_add(r_t[:], rs_a[:], rs_b[:])
                    nc.vector.reciprocal(r_t[:], r_t[:])
                    nc.vector.tensor_scalar_mul(row[:], row[:], r_t[:])
                    nc.sync.dma_start(
                        out=attn[mt * 128:(mt + 1) * 128, :], in_=row[:])

            # ---- P3: V' = atten_T^T @ V ----------------------------------
            for blk in range(NCH):
                if blk % 2 == 0:
                    pools = [pjps, pjps, scps, scps]
                    tags = ["pj", "pj", "sc", "sc"]
                else:
                    pools = [scps] * 4
                    tags = ["sc"] * 4
                vps = [
                    pools[j].tile([128, D], F32, tag=tags[j], name=f"vp{j}")
                    for j in range(4)
                ]
                for mq in range(MT // 4):
                    ats = []
                    for mi in range(4):
                        mt = mq * 4 + mi
                        at = atiles.tile([128, 512], F32R, tag="at", name="at")
                        nc.sync.dma_start(
                            out=at[:],
                            in_=attn[mt * 128:(mt + 1) * 128,
                                     blk * 512:(blk + 1) * 512].bitcast(F32R),
                        )
                        ats.append(at)
                    for j in range(4):
                        for mi in range(4):
                            mt = mq * 4 + mi
                            nc.tensor.matmul(
                                vps[j][:], ats[mi][:, j * 128:(j + 1) * 128],
                                v_t[:, mt, :],
                                start=(mt == 0), stop=(mt == MT - 1),
                            )
                for j in range(4):
                    stg = rows.tile([128, D], F32, tag="row", name="stg")
                    if j % 2 == 0:
                        nc.scalar.copy(stg[:], vps[j][:])
                    else:
                        nc.vector.tensor_copy(stg[:], vps[j][:])
                    nc.sync.dma_start(
                        out=vp[blk * 512 + j * 128: blk * 512 + (j + 1) * 128, :],
                        in_=stg[:],
                    )

    nc.compile()
    return nc


# ---------------------------------------------------------------------------
# Host entry point
# ---------------------------------------------------------------------------

_B, _S, _N, _D = 4, 512, 4096, 512
_MSH = _N // 2

_prog_cache = {}
_last_result = None


def _get_program(with_bias):
    key = (_S, _N, _D, _MSH, with_bias)
    if key not in _prog_cache:
        _prog_cache[key] = build_program(_S, _N, _D, _MSH, num_devices=8,
                                         with_bias=with_bias)
    return _prog_cache[key]


def kernel(x, adj, Wq, bq, Wk, bk, Wv, bv):
    from concourse import bass_utils

    x = np.asarray(x, dtype=np.float32)
    adj = np.asarray(adj)
    Wq = np.asarray(Wq, dtype=np.float32)
    Wk = np.asarray(Wk, dtype=np.float32)
    Wv = np.asarray(Wv, dtype=np.float32)
    bq = np.asarray(bq, dtype=np.float32)
    bk = np.asarray(bk, dtype=np.float32)
    bv = np.asarray(bv, dtype=np.float32)

    with_bias = bool(np.any(bq) or np.any(bk) or np.any(bv))
    nc = _get_program(with_bias)

    # adjT[m, n] = adj[n, m]
    adjm1_full = np.ascontiguousarray(adj.T.astype(np.int8) - np.int8(1))

    in_maps = []
    for c in range(8):
        b, h = divmod(c, 2)
        m0 = h * _MSH
        # Put this core's m-half first along the n axis of xb so the
        # program's "first MSH columns" are its m shard. The n axis of
        # adjm1 (and thus of attn/vp) is permuted identically; the host
        # undoes it on gather.
        xb = x[b]
        if h == 0:
            xb_perm = xb
            adj_part = adjm1_full[m0:m0 + _MSH, :]
        else:
            xb_perm = np.ascontiguousarray(
                np.concatenate([xb[:, m0:], xb[:, :m0]], axis=1))
            adj_part = np.ascontiguousarray(
                np.concatenate(
                    [adjm1_full[m0:, m0:], adjm1_full[m0:, :m0]], axis=1))
        in_maps.append({
            "xb": xb_perm,
            "wq": Wq, "wk": Wk, "wv": Wv,
            "bq": bq, "bk": bk, "bv": bv,
            "adjm1": adj_part,
        })

    res = bass_utils.run_bass_kernel_spmd(nc, in_maps, list(range(8)))
    global _last_result
    _last_result = res

    atten = np.empty((_B, _N, _N), dtype=np.float32)
    v_prime = np.empty((_B, _N, _D), dtype=np.float32)
    for b in range(_B):
        parts_t = []
        vp_sum = None
        for h in range(2):
            r = res.results[2 * b + h]
            a = r["attn"]  # [MSH, N] with permuted n axis
            v = r["vp"]    # [N, D] with permuted n axis
            if h == 1:
                a = np.concatenate([a[:, _MSH:], a[:, :_MSH]], axis=1)
                v = np.concatenate([v[_MSH:, :], v[:_MSH, :]], axis=0)
            parts_t.append(a)
            vp_sum = v.astype(np.float32) if vp_sum is None else vp_sum + v
        atten_t = np.concatenate(parts_t, axis=0)  # [N(m), N(n)]
        atten[b] = atten_t.T
        v_prime[b] = vp_sum
    return v_prime, atten


# revision 29
# speedup vs baseline: 1.1269x; 1.1269x over previous
"""Trainium2 Bass kernel for nn_AttenLayer (sparse graph attention).

Reference computation (B=4, S=512, N=4096, D=512):
    xt = x.transpose(0, 2, 1)                  # (B, N, S)
    Q = xt @ Wq + bq; K = xt @ Wk + bk; V = xt @ Wv + bv
    E = Q @ K^T                                # (B, N, N)
    E = where(adj > 0, E, -9e15)
    atten = softmax(E, axis=1)                 # over the query-node axis!
    V' = (atten @ V) / sqrt(512)

Sharding: 8 cores = 4 batches x 2 key-node (m) halves. Each core works in
the transposed score layout scoresT[m, n] = K @ Q^T so the softmax axis n
is the free axis:
    P1: QT[d, n] full and V[m_shard, d] projections (PE, fp32r);
        KT[d, m] is produced on the fly in P2 per 512-wide m chunk.
    P2 per m_tile: scoresT into PSUM; masked = scores + 9e15*(adjT-1)
        (DVE scalar_tensor_tensor, int8 mask); expE = exp(masked) with
        fused row sums (ACT accum_out, in place); atten_T = expE / rowsum
        (DVE/ACT split, in place); DMA out.
    P3: V'_partial = atten_T^T @ V_scaled, re-reading atten_T from HBM.
Host reassembles: atten[b] = vstack(two m-half atten_T parts).T and
V'[b] = sum of the two partial V'. The 1/sqrt(512) scale is folded into
the V projection on-chip. Each core receives xb with its own m-half
permuted to the front; the host undoes the n-axis permutation on gather.
"""

import math

import numpy as np

import concourse.bacc as bacc
import concourse.mybir as mybir
import concourse.tile as tile

F32 = mybir.dt.float32
F32R = mybir.dt.float32r
I8 = mybir.dt.int8
AF = mybir.ActivationFunctionType
ALU = mybir.AluOpType

NEGBIG = 9.0e15


def build_program(S=512, N=4096, D=512, MSH=2048, num_devices=8, with_bias=True):
    """One-core program; run SPMD on all cores with per-core input slices."""
    ST, DT = S // 128, D // 128
    NCH = N // 512          # n chunks of 512 (psum bank width)
    MT = MSH // 128         # m tiles
    MCH = MSH // 512        # m chunks (KT projection granularity)
    HALF = N // 2           # exp spans
    NRM_DVE = max(512, (3 * N // 8) // 512 * 512)  # DVE-normalized prefix
    VSCALE = 1.0 / math.sqrt(512.0)

    nc = bacc.Bacc("TRN2", target_bir_lowering=False, debug=False,
                   num_devices=num_devices)

    xb = nc.dram_tensor("xb", [S, N], F32R, kind="ExternalInput")
    wq = nc.dram_tensor("wq", [S, D], F32R, kind="ExternalInput")
    wk = nc.dram_tensor("wk", [S, D], F32R, kind="ExternalInput")
    wv = nc.dram_tensor("wv", [S, D], F32R, kind="ExternalInput")
    bq = nc.dram_tensor("bq", [D], F32, kind="ExternalInput")
    bk = nc.dram_tensor("bk", [D], F32, kind="ExternalInput")
    bv = nc.dram_tensor("bv", [D], F32, kind="ExternalInput")
    adjm1 = nc.dram_tensor("adjm1", [MSH, N], I8, kind="ExternalInput")
    attn = nc.dram_tensor("attn", [MSH, N], F32, kind="ExternalOutput")
    vp = nc.dram_tensor("vp", [N, D], F32, kind="ExternalOutput")

    with tile.TileContext(nc) as tc:
        with (
            tc.tile_pool(name="pers", bufs=1) as pers,
            tc.tile_pool(name="xbp", bufs=2) as xbp,
            tc.tile_pool(name="ktp", bufs=2) as ktp,
            tc.tile_pool(name="rows", bufs=2) as rows,
            tc.tile_pool(name="masks", bufs=2) as masks,
            tc.tile_pool(name="atiles", bufs=6) as atiles,
            tc.tile_pool(name="stats", bufs=6) as stats,
            tc.tile_pool(name="pjps", bufs=2, space="PSUM") as pjps,
            tc.tile_pool(name="scps", bufs=6, space="PSUM") as scps,
        ):
            # ---- constants / weights -------------------------------------
            wq_t = pers.tile([128, ST, D], F32R, tag="wq")
            wk_t = pers.tile([128, ST, D], F32R, tag="wk")
            wv_t = pers.tile([128, ST, D], F32R, tag="wv")
            nc.sync.dma_start(out=wq_t[:], in_=wq.rearrange("(st p) d -> p st d", p=128))
            nc.sync.dma_start(out=wk_t[:], in_=wk.rearrange("(st p) d -> p st d", p=128))
            nc.sync.dma_start(out=wv_t[:], in_=wv.rearrange("(st p) d -> p st d", p=128))
            if with_bias:
                bq_r = pers.tile([1, D], F32R, tag="bq")
                bk_r = pers.tile([1, D], F32R, tag="bk")
                bv_r = pers.tile([1, D], F32R, tag="bv")
                nc.sync.dma_start(out=bq_r[:], in_=bq[None, :].bitcast(F32R))
                nc.sync.dma_start(out=bk_r[:], in_=bk[None, :].bitcast(F32R))
                nc.sync.dma_start(out=bv_r[:], in_=bv[None, :].bitcast(F32R))
                ones_f = pers.tile([1, 512], F32, tag="ones_f")
                nc.vector.memset(ones_f[:], 1.0)
                ones_t = pers.tile([1, 128], F32R, tag="ones")
                nc.vector.tensor_copy(ones_t[:], ones_f[:, 0:128])
                ones512 = pers.tile([1, 512], F32R, tag="ones512")
                nc.vector.tensor_copy(ones512[:], ones_f[:])

            qt_t = pers.tile([128, DT, N], F32R, tag="qt")
            v_t = pers.tile([128, MT, D], F32R, tag="vt")

            # ---- P1: QT and V projections --------------------------------
            # QT[d, n] = Wq^T @ xb + bq: lhsT = wq[s, d], rhs = xb[s, n]
            for nch in range(NCH):
                xch = xbp.tile([128, ST, 512], F32R, tag="xch", name="xq")
                nc.sync.dma_start(
                    out=xch[:],
                    in_=xb[:, nch * 512:(nch + 1) * 512].rearrange(
                        "(st p) n -> p st n", p=128),
                )
                for dt in range(DT):
                    ps = pjps.tile([128, 512], F32, tag="pj", name="psq")
                    if with_bias:
                        nc.tensor.matmul(
                            ps[:], bq_r[0:1, dt * 128:(dt + 1) * 128],
                            ones512[0:1, :], start=True, stop=False,
                        )
                    for st in range(ST):
                        nc.tensor.matmul(
                            ps[:], wq_t[:, st, dt * 128:(dt + 1) * 128],
                            xch[:, st, :],
                            start=(st == 0 and not with_bias),
                            stop=(st == ST - 1),
                        )
                    nc.scalar.activation(
                        qt_t[:, dt, nch * 512:(nch + 1) * 512], ps[:], AF.Copy,
                    )

            # V[m, d] = (xb_m^T @ Wv + bv) / sqrt(512); m = first MSH cols
            for mch in range(MCH):
                xch = xbp.tile([128, ST, 512], F32R, tag="xch", name="xv")
                nc.sync.dma_start(
                    out=xch[:],
                    in_=xb[:, mch * 512:(mch + 1) * 512].rearrange(
                        "(st p) n -> p st n", p=128),
                )
                for mt_in in range(4):
                    mt = mch * 4 + mt_in
                    ps = pjps.tile([128, D], F32, tag="pj", name="psv")
                    if with_bias:
                        nc.tensor.matmul(
                            ps[:], ones_t[0:1, :], bv_r[0:1, :],
                            start=True, stop=False,
                        )
                    for st in range(ST):
                        nc.tensor.matmul(
                            ps[:], xch[:, st, mt_in * 128:(mt_in + 1) * 128],
                            wv_t[:, st, :],
                            start=(st == 0 and not with_bias),
                            stop=(st == ST - 1),
                        )
                    nc.scalar.activation(v_t[:, mt, :], ps[:], AF.Copy, scale=VSCALE)

            # ---- P2: scores -> masked softmax (m-outer) ------------------
            def kt_build(mch):
                # KT[d, m_chunk] = Wk^T @ xb_m + bk, produced per 512 chunk
                xch = xbp.tile([128, ST, 512], F32R, tag="xch", name="xk")
                nc.sync.dma_start(
                    out=xch[:],
                    in_=xb[:, mch * 512:(mch + 1) * 512].rearrange(
                        "(st p) n -> p st n", p=128),
                )
                kt_c = ktp.tile([128, DT, 512], F32R, tag="ktc", name="ktc")
                for dt in range(DT):
                    ps = pjps.tile([128, 512], F32, tag="pj", name="psk")
                    if with_bias:
                        nc.tensor.matmul(
                            ps[:], bk_r[0:1, dt * 128:(dt + 1) * 128],
                            ones512[0:1, :], start=True, stop=False,
                        )
                    for st in range(ST):
                        nc.tensor.matmul(
                            ps[:], wk_t[:, st, dt * 128:(dt + 1) * 128],
                            xch[:, st, :],
                            start=(st == 0 and not with_bias),
                            stop=(st == ST - 1),
                        )
                    nc.scalar.activation(kt_c[:, dt, :], ps[:], AF.Copy)
                return kt_c

            kt_cur = kt_build(0)
            for mch in range(MCH):
                kt_c = kt_cur
                for mt_in in range(4):
                    mt = mch * 4 + mt_in
                    if mt_in == 1 and mch + 1 < MCH:
                        kt_cur = kt_build(mch + 1)
                    msl = slice(mt_in * 128, (mt_in + 1) * 128)
                    mk = masks.tile([128, N], I8, tag="mk", name="mk")
                    nc.sync.dma_start(
                        out=mk[:], in_=adjm1[mt * 128:(mt + 1) * 128, :])
                    row = rows.tile([128, N], F32, tag="row", name="row")
                    for nch in range(NCH):
                        ps = scps.tile([128, 512], F32, tag="sc", name="sc")
                        for dt in range(DT):
                            nc.tensor.matmul(
                                ps[:], kt_c[:, dt, msl],
                                qt_t[:, dt, nch * 512:(nch + 1) * 512],
                                start=(dt == 0), stop=(dt == DT - 1),
                            )
                        nc.vector.scalar_tensor_tensor(
                            out=row[:, nch * 512:(nch + 1) * 512],
                            in0=mk[:, nch * 512:(nch + 1) * 512], scalar=NEGBIG,
                            in1=ps[:], op0=ALU.mult, op1=ALU.add,
                        )
                    rs_a = stats.tile([128, 1], F32, tag="rsa", name="rsa")
                    rs_b = stats.tile([128, 1], F32, tag="rsb", name="rsb")
                    nc.scalar.activation(
                        row[:, 0:HALF], row[:, 0:HALF], AF.Exp,
                        accum_out=rs_a[:],
                    )
                    nc.scalar.activation(
                        row[:, HALF:], row[:, HALF:], AF.Exp,
                        accum_out=rs_b[:],
                    )
                    r_t = stats.tile([128, 1], F32, tag="r", name="r")
                    nc.vector.tensor_add(r_t[:], rs_a[:], rs_b[:])
                    nc.vector.reciprocal(r_t[:], r_t[:])
                    nc.vector.tensor_scalar_mul(row[:], row[:], r_t[:])
                    nc.gpsimd.dma_start(
                        out=attn[mt * 128:(mt + 1) * 128, :], in_=row[:])

            # ---- P3: V' = atten_T^T @ V ----------------------------------
            for blk in range(NCH):
                if blk % 2 == 0:
                    pools = [pjps, pjps, scps, scps]
                    tags = ["pj", "pj", "sc", "sc"]
                else:
                    pools = [scps] * 4
                    tags = ["sc"] * 4
                vps = [
                    pools[j].tile([128, D], F32, tag=tags[j], name=f"vp{j}")
                    for j in range(4)
                ]
                for mt in range(MT):
                    at = atiles.tile([128, 512], F32R, tag="at", name="at")
                    nc.sync.dma_start(
                        out=at[:],
                        in_=attn[mt * 128:(mt + 1) * 128,
                                 blk * 512:(blk + 1) * 512].bitcast(F32R),
                    )
                    for j in range(4):
                        nc.tensor.matmul(
                            vps[j][:], at[:, j * 128:(j + 1) * 128],
                            v_t[:, mt, :],
                            start=(mt == 0), stop=(mt == MT - 1),
                        )
                for j in range(4):
                    stg = rows.tile([128, D], F32, tag="row", name="stg")
                    if j % 2 == 0:
                        nc.scalar.copy(stg[:], vps[j][:])
                    else:
                        nc.vector.tensor_copy(stg[:], vps[j][:])
                    nc.gpsimd.dma_start(
                        out=vp[blk * 512 + j * 128: blk * 512 + (j + 1) * 128, :],
                        in_=stg[:],
                    )

    nc.compile()
    return nc


# ---------------------------------------------------------------------------
# Host entry point
# ---------------------------------------------------------------------------

_B, _S, _N, _D = 4, 512, 4096, 512
_MSH = _N // 2

_prog_cache = {}
_last_result = None


def _get_program(with_bias):
    key = (_S, _N, _D, _MSH, with_bias)
    if key not in _prog_cache:
        _prog_cache[key] = build_program(_S, _N, _D, _MSH, num_devices=8,
                                         with_bias=with_bias)
    return _prog_cache[key]


def kernel(x, adj, Wq, bq, Wk, bk, Wv, bv):
    from concourse import bass_utils

    x = np.asarray(x, dtype=np.float32)
    adj = np.asarray(adj)
    Wq = np.asarray(Wq, dtype=np.float32)
    Wk = np.asarray(Wk, dtype=np.float32)
    Wv = np.asarray(Wv, dtype=np.float32)
    bq = np.asarray(bq, dtype=np.float32)
    bk = np.asarray(bk, dtype=np.float32)
    bv = np.asarray(bv, dtype=np.float32)

    with_bias = bool(np.any(bq) or np.any(bk) or np.any(bv))
    nc = _get_program(with_bias)

    # adjT[m, n] = adj[n, m]
    adjm1_full = np.ascontiguousarray(adj.T.astype(np.int8) - np.int8(1))

    in_maps = []
    for c in range(8):
        b, h = divmod(c, 2)
        m0 = h * _MSH
        # Put this core's m-half first along the n axis of xb so the
        # program's "first MSH columns" are its m shard. The n axis of
        # adjm1 (and thus of attn/vp) is permuted identically; the host
        # undoes it on gather.
        xb = x[b]
        if h == 0:
            xb_perm = xb
            adj_part = adjm1_full[m0:m0 + _MSH, :]
        else:
            xb_perm = np.ascontiguousarray(
                np.concatenate([xb[:, m0:], xb[:, :m0]], axis=1))
            adj_part = np.ascontiguousarray(
                np.concatenate(
                    [adjm1_full[m0:, m0:], adjm1_full[m0:, :m0]], axis=1))
        in_maps.append({
            "xb": xb_perm,
            "wq": Wq, "wk": Wk, "wv": Wv,
            "bq": bq, "bk": bk, "bv": bv,
            "adjm1": adj_part,
        })

    res = bass_utils.run_bass_kernel_spmd(nc, in_maps, list(range(8)))
    global _last_result
    _last_result = res

    atten = np.empty((_B, _N, _N), dtype=np.float32)
    v_prime = np.empty((_B, _N, _D), dtype=np.float32)
    for b in range(_B):
        parts_t = []
        vp_sum = None
        for h in range(2):
            r = res.results[2 * b + h]
            a = r["attn"]  # [MSH, N] with permuted n axis
            v = r["vp"]    # [N, D] with permuted n axis
            if h == 1:
                a = np.concatenate([a[:, _MSH:], a[:, :_MSH]], axis=1)
                v = np.concatenate([v[_MSH:, :], v[:_MSH, :]], axis=0)
            parts_t.append(a)
            vp_sum = v.astype(np.float32) if vp_sum is None else vp_sum + v
        atten_t = np.concatenate(parts_t, axis=0)  # [N(m), N(n)]
        atten[b] = atten_t.T
        v_prime[b] = vp_sum
    return v_prime, atten
